# revision 4
# baseline (speedup 1.0000x reference)
"""MoE kernel for Trainium2 (8 NeuronCores, expert-parallel).

Problem: nn_MoE_78151224918194
  hidden_states [4, 2048, 2048] f32 -> out [4, 2048, 2048] f32
  E=8 routed experts (top-2, softmax-renormalized), I=1408,
  plus a shared SwiGLU FFN with IS=2816.

Strategy:
  - Gate (softmax + top-2) computed on host with jax-on-CPU, exactly
    mirroring the reference ops, so expert selection matches bitwise.
  - Expert-parallel: core c runs expert c's FFN (f32r) over the tokens
    routed to it, host-gathered and padded to C = max expert count
    (exact, no rounding).
  - Shared FFN is token-parallel (1024 tokens/core) and computed with
    one-level Strassen in bf16: 7 products on (T/2, H/2, IS/2) blocks
    save 12.5% of PE cycles; block combines run on the otherwise-idle
    DVE/Act engines, weight combos are precomputed on host.
  - Host combine: y = shared slices, then y[idx_e] += w_e * yr_e.
"""

import os
import numpy as np
import ml_dtypes

import concourse.bacc as bacc
import concourse.mybir as mybir
import concourse.tile as tile
from concourse.bass_utils import run_bass_kernel_spmd

P = 128
H = 2048
I = 1408
E = 8
TOP_K = 2
IS = 2816
S_TOK = 1024        # shared tokens per core
KH = H // P         # 16 k-tiles over H
KI = I // P         # 11 k-tiles over I
KIS = IS // P       # 22 i-tiles over IS
F32 = mybir.dt.float32
F32R = mybir.dt.float32r
BF16 = mybir.dt.bfloat16
BF = ml_dtypes.bfloat16
SILU = mybir.ActivationFunctionType.Silu
COPY = mybir.ActivationFunctionType.Copy
ADD = mybir.AluOpType.add
SUB = mybir.AluOpType.subtract
MULT = mybir.AluOpType.mult

LAST_RESULTS = None  # BassKernelResults of the most recent device run
_BUILD_CACHE = {}


def _split_blocks(C):
    """Token blocks >=512 each (wg/wu stream coverage), preferring 1024."""
    blocks, rem = [], C
    while rem > 0:
        if rem >= 1536 or rem <= 1024:
            take = min(1024, rem)
        else:
            take = rem - 512
        blocks.append(take)
        rem -= take
    return blocks


def _split_subs(TB):
    """Near-equal moving sub-blocks, each even (fp32r ISA) and >=256."""
    n = max(1, (TB + 511) // 512)
    base = (TB // n) & ~1
    subs = [base] * n
    extra = TB - base * n
    assert extra % 2 == 0
    for j in range(extra // 2):
        subs[j % n] += 2
    return subs


def _emit_ffn_routed(nc, sbuf, psum, x_ap, wg_ap, wu_ap, wd_ap, out_ap,
                     n_tok, jtag):
    """Direct SwiGLU FFN (I=1408, f32r) over n_tok tokens (baseline path).

    x_ap:   DRAM [H, n_tok]   (tokens transposed)
    wg/wu:  DRAM [KI, P, KH, P]  (i-tile, partition(H), k-tile, I-cols)
    wd:     DRAM [KH, P, KI, P]  (m-tile, partition(I), i-tile, H-cols)
    out_ap: DRAM [H, n_tok]   out = ((silu(x@wg) * (x@wu)) @ wd).T
    """
    b0 = 0
    for TB in _split_blocks(n_tok):
        subs = []
        s = 0
        for w in _split_subs(TB):
            subs.append((s, w))
            s += w
        bt = f"{jtag}b{b0}"

        x_tiles = []
        for k in range(KH):
            xt = sbuf.tile([P, TB], F32R, name=f"x{bt}k{k}", tag=f"x{k}",
                           bufs=1)
            nc.sync.dma_start(xt[:], x_ap[k * P:(k + 1) * P, b0:b0 + TB])
            x_tiles.append(xt)

        act_tiles = []
        for i in range(KI):
            wg_sb = sbuf.tile([P, KH, P], F32R, name=f"wg{bt}i{i}", tag="wg",
                              bufs=2)
            wu_sb = sbuf.tile([P, KH, P], F32R, name=f"wu{bt}i{i}", tag="wu",
                              bufs=2)
            nc.sync.dma_start(wg_sb[:], wg_ap[i])
            nc.sync.dma_start(wu_sb[:], wu_ap[i])
            act = sbuf.tile([P, TB], F32R, name=f"act{bt}i{i}", tag=f"act{i}",
                            bufs=1)
            for s, w in subs:
                pg = psum.tile([P, 512], F32, name=f"pg{bt}i{i}s{s}", tag="pg",
                               bufs=2)
                pu = psum.tile([P, 512], F32, name=f"pu{bt}i{i}s{s}", tag="pu",
                               bufs=2)
                for k in range(KH):
                    nc.tensor.matmul(
                        pg[:, :w], wg_sb[:, k], x_tiles[k][:, s:s + w],
                        start=(k == 0), stop=(k == KH - 1))
                for k in range(KH):
                    nc.tensor.matmul(
                        pu[:, :w], wu_sb[:, k], x_tiles[k][:, s:s + w],
                        start=(k == 0), stop=(k == KH - 1))
                tmp = sbuf.tile([P, 512], F32, name=f"tmp{bt}i{i}s{s}",
                                tag="silu", bufs=3)
                nc.scalar.activation(tmp[:, :w], pg[:, :w], SILU)
                nc.vector.tensor_tensor(act[:, s:s + w], tmp[:, :w],
                                        pu[:, :w], MULT)
            act_tiles.append(act)

        for m in range(KH):
            wd_sb = sbuf.tile([P, KI, P], F32R, name=f"wd{bt}m{m}", tag="wd",
                              bufs=2)
            nc.sync.dma_start(wd_sb[:], wd_ap[m])
            for s, w in subs:
                po = psum.tile([P, 512], F32, name=f"po{bt}m{m}s{s}", tag="po",
                               bufs=2)
                for i in range(KI):
                    nc.tensor.matmul(
                        po[:, :w], wd_sb[:, i], act_tiles[i][:, s:s + w],
                        start=(i == 0), stop=(i == KI - 1))
                ot = sbuf.tile([P, 512], F32, name=f"ot{bt}m{m}s{s}", tag="ot",
                               bufs=3)
                nc.vector.tensor_copy(ot[:, :w], po[:, :w])
                nc.sync.dma_start(out_ap[m * P:(m + 1) * P, b0 + s:b0 + s + w],
                                  ot[:, :w])
        b0 += TB


# Strassen: M1=(A11+A22)(B11+B22), M2=(A21+A22)B11, M3=A11(B12-B22),
# M4=A22(B21-B11), M5=(A11+A12)B22, M6=(A21-A11)(B11+B12),
# M7=(A12-A22)(B21+B22)
# C11=M1+M4-M5+M7, C12=M3+M5, C21=M2+M4, C22=M1-M2+M3+M6
#
# Incremental combine schedule (product index p -> list of (quad, op)):
#   quad order: 0=C11, 1=C12, 2=C21, 3=C22; op: None=copy, ADD, SUB
_COMBINE = {
    0: [(0, None), (3, None)],
    1: [(2, None), (3, SUB)],
    2: [(1, None), (3, ADD)],
    3: [(0, ADD), (2, ADD)],
    4: [(0, SUB), (1, ADD)],
    5: [(3, ADD)],
    6: [(0, ADD)],
}


def _emit_shared_strassen(nc, tc, x_ap, wcg_ap, wcu_ap, wcd_ap, out_ap, jtag):
    """Shared SwiGLU FFN (IS=2816) over S_TOK tokens, bf16 Strassen.

    x_ap:  DRAM [P, KH, S_TOK] bf16 (partition, k-tile, token)
    wcg/wcu: DRAM [7, KIS//2, P, KH//2, P] bf16 (product, i-tile, part, k, col)
    wcd:   DRAM [7, KH//2, P, KIS//2, P] bf16 (product, m-tile, part, k, col)
    out_ap: DRAM [H, S_TOK] f32
    """
    TH = S_TOK // 2   # 512: token half
    KH2 = KH // 2     # 8
    KIS2 = KIS // 2   # 11

    with tc.tile_pool(name=f"shA{jtag}", bufs=1) as pA:
        # h [P, KIS, S_TOK] bf16: i-tiles 0..10 = IS1, 11..21 = IS2;
        # cols 0:TH = T1, TH: = T2
        ht = pA.tile([P, KIS, S_TOK], BF16, name=f"h{jtag}", tag="hsh")

        with tc.tile_pool(name=f"shB{jtag}", bufs=1) as pB:
            xt = pB.tile([P, KH, S_TOK], BF16, name=f"xs{jtag}", tag="xs")
            nc.sync.dma_start(xt[:], x_ap)

            A11 = xt[:, 0:KH2, 0:TH]
            A12 = xt[:, KH2:KH, 0:TH]
            A21 = xt[:, 0:KH2, TH:S_TOK]
            A22 = xt[:, KH2:KH, TH:S_TOK]
            mv_specs = [(A11, A22, ADD), (A21, A22, ADD), None, None,
                        (A11, A12, ADD), (A21, A11, SUB), (A12, A22, SUB)]
            movs = []
            for p, spec in enumerate(mv_specs):
                if spec is None:
                    movs.append(A11 if p == 2 else A22)
                    continue
                mt = pB.tile([P, KH2, TH], BF16, name=f"mv{jtag}{p}",
                             tag=f"mv{p}")
                nc.vector.tensor_tensor(mt[:], spec[0], spec[1], spec[2])
                movs.append(mt)

            with tc.tile_pool(name=f"shBp{jtag}", bufs=1, space="PSUM") as pp:
                for i in range(KIS2):
                    quads = {}
                    for gu, w_ap in (("g", wcg_ap), ("u", wcu_ap)):
                        qt = [pB.tile([P, TH], F32,
                                      name=f"q{jtag}{gu}{i}q{q}",
                                      tag=f"q{gu}{q}", bufs=2)
                              for q in range(4)]
                        for p in range(7):
                            wt = pB.tile([P, KH2, P], BF16,
                                         name=f"w{jtag}{gu}{i}p{p}",
                                         tag=f"w{p}", bufs=2)
                            nc.sync.dma_start(wt[:], w_ap[p, i])
                            mp = pp.tile([P, TH], F32,
                                         name=f"mp{jtag}{gu}{i}p{p}",
                                         tag="mp", bufs=3)
                            for k in range(KH2):
                                nc.tensor.matmul(
                                    mp[:], wt[:, k], movs[p][:, k, :],
                                    start=(k == 0), stop=(k == KH2 - 1))
                            for q, op in _COMBINE[p]:
                                if op is None:
                                    nc.scalar.activation(qt[q][:], mp[:], COPY)
                                else:
                                    nc.vector.tensor_tensor(qt[q][:], qt[q][:],
                                                            mp[:], op)
                        quads[gu] = qt
                    # h = silu(g) * u per quadrant -> bf16 into ht
                    for q in range(4):
                        row = i if q in (0, 2) else i + KIS2
                        col = slice(0, TH) if q in (0, 1) else slice(TH, S_TOK)
                        tmp = pB.tile([P, TH], F32, name=f"t{jtag}{i}q{q}",
                                      tag="silu", bufs=3)
                        nc.scalar.activation(tmp[:], quads["g"][q][:], SILU)
                        nc.vector.tensor_tensor(ht[:, row, col], tmp[:],
                                                quads["u"][q][:], MULT)

        with tc.tile_pool(name=f"shC{jtag}", bufs=1) as pC:
            D11 = ht[:, 0:KIS2, 0:TH]
            D12 = ht[:, KIS2:KIS, 0:TH]
            D21 = ht[:, 0:KIS2, TH:S_TOK]
            D22 = ht[:, KIS2:KIS, TH:S_TOK]
            dmv_specs = [(D11, D22, ADD), (D21, D22, ADD), None, None,
                         (D11, D12, ADD), (D21, D11, SUB), (D12, D22, SUB)]
            dmovs = []
            for p, spec in enumerate(dmv_specs):
                if spec is None:
                    dmovs.append(D11 if p == 2 else D22)
                    continue
                mt = pC.tile([P, KIS2, TH], BF16, name=f"dmv{jtag}{p}",
                             tag=f"dmv{p}")
                nc.vector.tensor_tensor(mt[:], spec[0], spec[1], spec[2])
                dmovs.append(mt)

            with tc.tile_pool(name=f"shCp{jtag}", bufs=1, space="PSUM") as pp:
                for m in range(KH2):
                    yq = [pC.tile([P, TH], F32, name=f"y{jtag}{m}q{q}",
                                  tag=f"yq{q}", bufs=2) for q in range(4)]
                    for p in range(7):
                        wt = pC.tile([P, KIS2, P], BF16,
                                     name=f"wd{jtag}{m}p{p}", tag=f"wd{p}",
                                     bufs=2)
                        nc.sync.dma_start(wt[:], wcd_ap[p, m])
                        mp = pp.tile([P, TH], F32, name=f"md{jtag}{m}p{p}",
                                     tag="md", bufs=3)
                        for k in range(KIS2):
                            nc.tensor.matmul(
                                mp[:], wt[:, k], dmovs[p][:, k, :],
                                start=(k == 0), stop=(k == KIS2 - 1))
                        for q, op in _COMBINE[p]:
                            if op is None:
                                nc.scalar.activation(yq[q][:], mp[:], COPY)
                            else:
                                nc.vector.tensor_tensor(yq[q][:], yq[q][:],
                                                        mp[:], op)
                    # C11=y[T1,H1] C12=y[T1,H2] C21=y[T2,H1] C22=y[T2,H2]
                    for q in range(4):
                        row = m if q in (0, 2) else m + KH2
                        c0 = 0 if q in (0, 1) else TH
                        nc.sync.dma_start(
                            out_ap[row * P:(row + 1) * P, c0:c0 + TH],
                            yq[q][:])


def _build(C, reps=1, loop=0):
    nc = bacc.Bacc(trn_type="TRN2", target_bir_lowering=False, debug=False)
    xr = nc.dram_tensor("xr", [H, C], F32R, kind="ExternalInput")
    xs = nc.dram_tensor("xs", [P, KH, S_TOK], BF16, kind="ExternalInput")
    w_in = {}
    for nm in ("rg", "ru"):
        w_in[nm] = nc.dram_tensor(nm, [KI, P, KH, P], F32R,
                                  kind="ExternalInput")
    w_in["rd"] = nc.dram_tensor("rd", [KH, P, KI, P], F32R,
                                kind="ExternalInput")
    for nm in ("cg", "cu"):
        w_in[nm] = nc.dram_tensor(nm, [7, KIS // 2, P, KH // 2, P], BF16,
                                  kind="ExternalInput")
    w_in["cd"] = nc.dram_tensor("cd", [7, KH // 2, P, KIS // 2, P], BF16,
                                kind="ExternalInput")
    yr = nc.dram_tensor("yr", [H, C], F32, kind="ExternalOutput")
    ys = nc.dram_tensor("ys", [H, S_TOK], F32, kind="ExternalOutput")

    with tile.TileContext(nc) as tc:
        def body(r):
            with (
                tc.tile_pool(name=f"sbufr{r}", bufs=2) as sbuf,
                tc.tile_pool(name=f"psumr{r}", bufs=2, space="PSUM") as psum,
            ):
                _emit_ffn_routed(nc, sbuf, psum, xr.ap(), w_in["rg"].ap(),
                                 w_in["ru"].ap(), w_in["rd"].ap(), yr.ap(),
                                 C, f"r{r}_")
            _emit_shared_strassen(nc, tc, xs.ap(), w_in["cg"].ap(),
                                  w_in["cu"].ap(), w_in["cd"].ap(), ys.ap(),
                                  f"s{r}_")

        if loop:
            with tc.For_i(0, loop, 1):
                for r in range(reps):
                    body(r)
        else:
            for r in range(reps):
                body(r)
    nc.compile()
    return nc


def _get_nc(C, reps=1, loop=0):
    key = (C, reps, loop)
    if key not in _BUILD_CACHE:
        _BUILD_CACHE[key] = _build(C, reps, loop)
    return _BUILD_CACHE[key]


def _gate_host(x, gate_w):
    """Softmax + top-2 + renormalize, mirroring the jax reference on CPU."""
    try:
        import jax
        import jax.numpy as jnp
        cpu = jax.devices("cpu")[0]
        with jax.default_device(cpu):
            logits = jnp.asarray(x) @ jnp.asarray(gate_w).T
            scores = jax.nn.softmax(logits, axis=-1)
            topk_w, topk_idx = jax.lax.top_k(scores, TOP_K)
            topk_w = topk_w / (jnp.sum(topk_w, axis=-1, keepdims=True) + 1e-20)
        return np.asarray(topk_w), np.asarray(topk_idx)
    except Exception:
        logits = x @ gate_w.T
        m = logits.max(axis=-1, keepdims=True)
        ex = np.exp(logits - m)
        scores = ex / ex.sum(axis=-1, keepdims=True)
        order = np.argsort(-scores, axis=-1, kind="stable")
        topk_idx = order[:, :TOP_K]
        topk_w = np.take_along_axis(scores, topk_idx, axis=-1)
        topk_w = topk_w / (topk_w.sum(axis=-1, keepdims=True) + 1e-20)
        return topk_w.astype(np.float32), topk_idx.astype(np.int32)


def _wlayout_ud(w):
    # [H, I_like] -> [KI', P(H), KH, P(I)]  (stationary tiles for up/gate)
    ki = w.shape[1] // P
    return np.ascontiguousarray(w.reshape(KH, P, ki, P).transpose(2, 1, 0, 3))


def _wlayout_down(w):
    # [I_like, H] -> [KH, P(I), KI', P(H)]
    ki = w.shape[0] // P
    return np.ascontiguousarray(w.reshape(ki, P, KH, P).transpose(2, 1, 0, 3))


def _strassen_combos(B11, B12, B21, B22):
    return [B11 + B22, B11, B12 - B22, B21 - B11, B22, B11 + B12, B21 + B22]


def _wlayout_strassen_ud(w):
    """[H, IS] -> [7, KIS/2, P, KH/2, P] bf16 weight-combo tensor."""
    h2, n2 = H // 2, IS // 2
    combos = _strassen_combos(w[:h2, :n2], w[:h2, n2:], w[h2:, :n2],
                              w[h2:, n2:])
    out = np.empty((7, n2 // P, P, h2 // P, P), dtype=BF)
    for p, cb in enumerate(combos):
        out[p] = cb.reshape(h2 // P, P, n2 // P, P).transpose(
            2, 1, 0, 3).astype(BF)
    return np.ascontiguousarray(out)


def _wlayout_strassen_down(w):
    """[IS, H] -> [7, KH/2, P, KIS/2, P] bf16 weight-combo tensor."""
    k2, m2 = IS // 2, H // 2
    combos = _strassen_combos(w[:k2, :m2], w[:k2, m2:], w[k2:, :m2],
                              w[k2:, m2:])
    out = np.empty((7, m2 // P, P, k2 // P, P), dtype=BF)
    for p, cb in enumerate(combos):
        out[p] = cb.reshape(k2 // P, P, m2 // P, P).transpose(
            2, 1, 0, 3).astype(BF)
    return np.ascontiguousarray(out)


def _prepare(hidden_states, gate_w, we_gate, we_up, we_down,
             ws_gate, ws_up, ws_down):
    B, S, h = hidden_states.shape
    x = np.ascontiguousarray(hidden_states.reshape(-1, h))  # [T, H]

    topk_w, topk_idx = _gate_host(x, gate_w)

    idx_lists, w_lists = [], []
    for e in range(E):
        mask = (topk_idx == e)
        idx = np.nonzero(mask.any(axis=1))[0]
        we = np.where(mask, topk_w, 0.0).sum(axis=1)[idx].astype(np.float32)
        idx_lists.append(idx)
        w_lists.append(we)
    C = max(512, (max(len(ix) for ix in idx_lists) + 3) & ~3)

    cg = _wlayout_strassen_ud(ws_gate)
    cu = _wlayout_strassen_ud(ws_up)
    cd = _wlayout_strassen_down(ws_down)

    in_maps = []
    for c in range(E):
        idx = idx_lists[c]
        xr = np.zeros((H, C), dtype=np.float32)
        xr[:, :len(idx)] = x[idx].T
        xs = np.ascontiguousarray(
            x[c * S_TOK:(c + 1) * S_TOK].T.reshape(KH, P, S_TOK).transpose(
                1, 0, 2).astype(BF))
        in_maps.append({
            "xr": xr, "xs": xs,
            "rg": _wlayout_ud(we_gate[c]),
            "ru": _wlayout_ud(we_up[c]),
            "rd": _wlayout_down(we_down[c]),
            "cg": cg, "cu": cu, "cd": cd,
        })
    return in_maps, idx_lists, w_lists, C


def _combine(results, idx_lists, w_lists, T):
    y = np.empty((T, H), dtype=np.float32)
    for c in range(E):
        y[c * S_TOK:(c + 1) * S_TOK] = results[c]["ys"].T
    for c in range(E):
        idx = idx_lists[c]
        y[idx] += w_lists[c][:, None] * results[c]["yr"][:, :len(idx)].T
    return y


def kernel(hidden_states, gate_w, we_gate, we_up, we_down,
           ws_gate, ws_up, ws_down):
    global LAST_RESULTS
    B, S, h = hidden_states.shape
    in_maps, idx_lists, w_lists, C = _prepare(
        hidden_states, gate_w, we_gate, we_up, we_down,
        ws_gate, ws_up, ws_down)

    nc = _get_nc(C)

    trace_env = os.environ.get("MOE_TRACE", "")
    kwargs = {}
    if trace_env:
        kwargs["trace"] = True
        kwargs["trace_cores"] = [int(t) for t in trace_env.split(",")] \
            if trace_env != "1" else [0]
    res = run_bass_kernel_spmd(nc, in_maps, core_ids=list(range(E)), **kwargs)
    LAST_RESULTS = res

    y = _combine(res.results, idx_lists, w_lists, B * S)
    return y.reshape(B, S, h)


# revision 16
# speedup vs baseline: 1.8039x; 1.8039x over previous
"""MoE kernel for Trainium2 (8 NeuronCores, expert-parallel).

Problem: nn_MoE_78151224918194
  hidden_states [4, 2048, 2048] f32 -> out [4, 2048, 2048] f32
  E=8 routed experts (top-2, softmax-renormalized), I=1408,
  plus a shared SwiGLU FFN with IS=2816.

Strategy:
  - Gate (softmax + top-2) computed on host with jax-on-CPU, exactly
    mirroring the reference ops, so expert selection matches bitwise.
  - Expert-parallel: core c runs expert c's FFN (f32r) over the tokens
    routed to it, host-gathered and padded to C = max expert count
    (exact, no rounding).
  - Shared FFN is token-parallel (1024 tokens/core) and computed with
    one-level Strassen in bf16: 7 products on (T/2, H/2, IS/2) blocks
    save 12.5% of PE cycles; block combines run on the otherwise-idle
    DVE/Act engines, weight combos are precomputed on host.
  - Host combine: y = shared slices, then y[idx_e] += w_e * yr_e.
"""

import os
import numpy as np
import ml_dtypes

import concourse.bacc as bacc
import concourse.mybir as mybir
import concourse.tile as tile
from concourse.bass_utils import run_bass_kernel_spmd

P = 128
H = 2048
I = 1408
E = 8
TOP_K = 2
IS = 2816
S_TOK = 1024        # shared tokens per core
KH = H // P         # 16 k-tiles over H
KI = I // P         # 11 k-tiles over I
KIS = IS // P       # 22 i-tiles over IS
F32 = mybir.dt.float32
F32R = mybir.dt.float32r
BF16 = mybir.dt.bfloat16
BF = ml_dtypes.bfloat16
SILU = mybir.ActivationFunctionType.Silu
COPY = mybir.ActivationFunctionType.Copy
ADD = mybir.AluOpType.add
SUB = mybir.AluOpType.subtract
MULT = mybir.AluOpType.mult

LAST_RESULTS = None  # BassKernelResults of the most recent device run
_BUILD_CACHE = {}


def _split_blocks(C):
    """Token blocks: as few as possible (weights re-stream per block) while
    keeping x+act SBUF for a block bounded (<=1088 tokens)."""
    nb = max(1, (C + 1087) // 1088)
    base = (C // nb) & ~1
    blocks = [base] * nb
    extra = C - base * nb
    assert extra % 2 == 0
    for j in range(extra // 2):
        blocks[j % nb] += 2
    return blocks


def _split_subs(TB):
    """Near-equal moving sub-blocks, each even (fp32r ISA) and >=256."""
    n = max(1, (TB + 511) // 512)
    base = (TB // n) & ~1
    subs = [base] * n
    extra = TB - base * n
    assert extra % 2 == 0
    for j in range(extra // 2):
        subs[j % n] += 2
    return subs


def _emit_routed_bf16(nc, sbuf, psum, x_ap, wg_ap, wu_ap, wd_ap, out_ap,
                      n_tok, jtag):
    """Single-pass SwiGLU FFN (I=1408, bf16): weights stream exactly once,
    all n_tok tokens resident in SBUF.

    x_ap:   DRAM [P, KH, n_tok] bf16
    wg/wu:  DRAM [KI, P, KH, P] bf16; wd: DRAM [KH, P, KI, P] bf16
    out_ap: DRAM [H, n_tok] f32
    """
    subs = []
    s = 0
    for w in _split_subs_512(n_tok):
        subs.append((s, w))
        s += w

    xt = sbuf.tile([P, KH, n_tok], BF16, name=f"x{jtag}", tag="xr", bufs=1)
    for s, w in subs:
        nc.sync.dma_start(xt[:, :, s:s + w], x_ap[:, :, s:s + w])
    ht = sbuf.tile([P, KI, n_tok], BF16, name=f"h{jtag}", tag="actr", bufs=1)

    for i in range(KI):
        wg_sb = sbuf.tile([P, KH, P], BF16, name=f"wg{jtag}i{i}", tag="wg",
                          bufs=2)
        wu_sb = sbuf.tile([P, KH, P], BF16, name=f"wu{jtag}i{i}", tag="wu",
                          bufs=2)
        nc.sync.dma_start(wg_sb[:], wg_ap[i])
        nc.sync.dma_start(wu_sb[:], wu_ap[i])
        for s, w in subs:
            pg = psum.tile([P, 512], F32, name=f"pg{jtag}i{i}s{s}", tag="pg",
                           bufs=2)
            pu = psum.tile([P, 512], F32, name=f"pu{jtag}i{i}s{s}", tag="pu",
                           bufs=2)
            for k in range(KH):
                nc.tensor.matmul(pg[:, :w], wg_sb[:, k], xt[:, k, s:s + w],
                                 start=(k == 0), stop=(k == KH - 1))
            for k in range(KH):
                nc.tensor.matmul(pu[:, :w], wu_sb[:, k], xt[:, k, s:s + w],
                                 start=(k == 0), stop=(k == KH - 1))
            tmp = sbuf.tile([P, 512], F32, name=f"tmp{jtag}i{i}s{s}",
                            tag="silu", bufs=3)
            nc.scalar.activation(tmp[:, :w], pg[:, :w], SILU)
            nc.vector.tensor_tensor(ht[:, i, s:s + w], tmp[:, :w],
                                    pu[:, :w], MULT)

    for m in range(KH):
        wd_sb = sbuf.tile([P, KI, P], BF16, name=f"wd{jtag}m{m}", tag="wd",
                          bufs=2)
        nc.sync.dma_start(wd_sb[:], wd_ap[m])
        for s, w in subs:
            po = psum.tile([P, 512], F32, name=f"po{jtag}m{m}s{s}", tag="po",
                           bufs=2)
            for i in range(KI):
                nc.tensor.matmul(po[:, :w], wd_sb[:, i], ht[:, i, s:s + w],
                                 start=(i == 0), stop=(i == KI - 1))
            ot = sbuf.tile([P, 512], F32, name=f"ot{jtag}m{m}s{s}", tag="ot",
                           bufs=3)
            nc.vector.tensor_copy(ot[:, :w], po[:, :w])
            nc.sync.dma_start(out_ap[m * P:(m + 1) * P, s:s + w], ot[:, :w])


def _split_subs_512(n_tok):
    """Near-equal sub-blocks, each <=512 (PSUM bank)."""
    n = max(1, (n_tok + 511) // 512)
    base, r = divmod(n_tok, n)
    return [base + (1 if j < r else 0) for j in range(n)]


def _emit_ffn_routed(nc, sbuf, psum, x_ap, wg_ap, wu_ap, wd_ap, out_ap,
                     n_tok, jtag):
    """Direct SwiGLU FFN (I=1408, f32r) over n_tok tokens (baseline path).

    x_ap:   DRAM [H, n_tok]   (tokens transposed)
    wg/wu:  DRAM [KI, P, KH, P]  (i-tile, partition(H), k-tile, I-cols)
    wd:     DRAM [KH, P, KI, P]  (m-tile, partition(I), i-tile, H-cols)
    out_ap: DRAM [H, n_tok]   out = ((silu(x@wg) * (x@wu)) @ wd).T
    """
    b0 = 0
    for TB in _split_blocks(n_tok):
        subs = []
        s = 0
        for w in _split_subs(TB):
            subs.append((s, w))
            s += w
        bt = f"{jtag}b{b0}"

        x_tiles = []
        for k in range(KH):
            xt = sbuf.tile([P, TB], F32R, name=f"x{bt}k{k}", tag=f"x{k}",
                           bufs=1)
            nc.sync.dma_start(xt[:], x_ap[k * P:(k + 1) * P, b0:b0 + TB])
            x_tiles.append(xt)

        act_tiles = []
        for i in range(KI):
            wg_sb = sbuf.tile([P, KH, P], F32R, name=f"wg{bt}i{i}", tag="wg",
                              bufs=2)
            wu_sb = sbuf.tile([P, KH, P], F32R, name=f"wu{bt}i{i}", tag="wu",
                              bufs=2)
            nc.sync.dma_start(wg_sb[:], wg_ap[i])
            nc.sync.dma_start(wu_sb[:], wu_ap[i])
            act = sbuf.tile([P, TB], F32R, name=f"act{bt}i{i}", tag=f"act{i}",
                            bufs=1)
            for s, w in subs:
                pg = psum.tile([P, 512], F32, name=f"pg{bt}i{i}s{s}", tag="pg",
                               bufs=2)
                pu = psum.tile([P, 512], F32, name=f"pu{bt}i{i}s{s}", tag="pu",
                               bufs=2)
                for k in range(KH):
                    nc.tensor.matmul(
                        pg[:, :w], wg_sb[:, k], x_tiles[k][:, s:s + w],
                        start=(k == 0), stop=(k == KH - 1))
                for k in range(KH):
                    nc.tensor.matmul(
                        pu[:, :w], wu_sb[:, k], x_tiles[k][:, s:s + w],
                        start=(k == 0), stop=(k == KH - 1))
                tmp = sbuf.tile([P, 512], F32, name=f"tmp{bt}i{i}s{s}",
                                tag="silu", bufs=3)
                nc.scalar.activation(tmp[:, :w], pg[:, :w], SILU)
                nc.vector.tensor_tensor(act[:, s:s + w], tmp[:, :w],
                                        pu[:, :w], MULT)
            act_tiles.append(act)

        for m in range(KH):
            wd_sb = sbuf.tile([P, KI, P], F32R, name=f"wd{bt}m{m}", tag="wd",
                              bufs=2)
            nc.sync.dma_start(wd_sb[:], wd_ap[m])
            for s, w in subs:
                po = psum.tile([P, 512], F32, name=f"po{bt}m{m}s{s}", tag="po",
                               bufs=2)
                for i in range(KI):
                    nc.tensor.matmul(
                        po[:, :w], wd_sb[:, i], act_tiles[i][:, s:s + w],
                        start=(i == 0), stop=(i == KI - 1))
                ot = sbuf.tile([P, 512], F32, name=f"ot{bt}m{m}s{s}", tag="ot",
                               bufs=3)
                nc.vector.tensor_copy(ot[:, :w], po[:, :w])
                nc.sync.dma_start(out_ap[m * P:(m + 1) * P, b0 + s:b0 + s + w],
                                  ot[:, :w])
        b0 += TB


# Strassen: M1=(A11+A22)(B11+B22), M2=(A21+A22)B11, M3=A11(B12-B22),
# M4=A22(B21-B11), M5=(A11+A12)B22, M6=(A21-A11)(B11+B12),
# M7=(A12-A22)(B21+B22)
# C11=M1+M4-M5+M7, C12=M3+M5, C21=M2+M4, C22=M1-M2+M3+M6
#
# Incremental combine schedule (product index p -> list of (quad, op)):
#   quad order: 0=C11, 1=C12, 2=C21, 3=C22; op: None=copy, ADD, SUB
_COMBINE = {
    0: [(0, None), (3, None)],
    1: [(2, None), (3, SUB)],
    2: [(1, None), (3, ADD)],
    3: [(0, ADD), (2, ADD)],
    4: [(0, SUB), (1, ADD)],
    5: [(3, ADD)],
    6: [(0, ADD)],
}


def _emit_shared_strassen(nc, tc, x_ap, wcg_ap, wcu_ap, wcd_ap, out_ap, jtag):
    """Shared SwiGLU FFN (IS=2816) over S_TOK tokens, bf16 Strassen.

    x_ap:  DRAM [P, KH, S_TOK] bf16 (partition, k-tile, token)
    wcg/wcu: DRAM [7, KIS//2, P, KH//2, P] bf16 (product, i-tile, part, k, col)
    wcd:   DRAM [7, KH//2, P, KIS//2, P] bf16 (product, m-tile, part, k, col)
    out_ap: DRAM [H, S_TOK] f32
    """
    TH = S_TOK // 2   # 512: token half
    KH2 = KH // 2     # 8
    KIS2 = KIS // 2   # 11

    with tc.tile_pool(name=f"shA{jtag}", bufs=1) as pA:
        # h [P, KIS, S_TOK] bf16: i-tiles 0..10 = IS1, 11..21 = IS2;
        # cols 0:TH = T1, TH: = T2
        ht = pA.tile([P, KIS, S_TOK], BF16, name=f"h{jtag}", tag="hsh")

        with tc.tile_pool(name=f"shB{jtag}", bufs=1) as pB:
            xt = pB.tile([P, KH, S_TOK], BF16, name=f"xs{jtag}", tag="xs")
            nc.sync.dma_start(xt[:], x_ap)

            A11 = xt[:, 0:KH2, 0:TH]
            A12 = xt[:, KH2:KH, 0:TH]
            A21 = xt[:, 0:KH2, TH:S_TOK]
            A22 = xt[:, KH2:KH, TH:S_TOK]
            mv_specs = [(A11, A22, ADD), (A21, A22, ADD), None, None,
                        (A11, A12, ADD), (A21, A11, SUB), (A12, A22, SUB)]
            movs = []
            for p, spec in enumerate(mv_specs):
                if spec is None:
                    movs.append(A11 if p == 2 else A22)
                    continue
                mt = pB.tile([P, KH2, TH], BF16, name=f"mv{jtag}{p}",
                             tag=f"mv{p}")
                nc.vector.tensor_tensor(mt[:], spec[0], spec[1], spec[2])
                movs.append(mt)

            with tc.tile_pool(name=f"shBp{jtag}", bufs=1, space="PSUM") as pp:
                for i in range(KIS2):
                    quads = {}
                    for gu, w_ap in (("g", wcg_ap), ("u", wcu_ap)):
                        qt = [pB.tile([P, TH], F32,
                                      name=f"q{jtag}{gu}{i}q{q}",
                                      tag=f"q{gu}{q}", bufs=2)
                              for q in range(4)]
                        for p in range(7):
                            wt = pB.tile([P, KH2, P], BF16,
                                         name=f"w{jtag}{gu}{i}p{p}",
                                         tag=f"w{p}", bufs=2)
                            nc.sync.dma_start(wt[:], w_ap[p, i])
                            mp = pp.tile([P, TH], F32,
                                         name=f"mp{jtag}{gu}{i}p{p}",
                                         tag="mp", bufs=6)
                            for k in range(KH2):
                                nc.tensor.matmul(
                                    mp[:], wt[:, k], movs[p][:, k, :],
                                    start=(k == 0), stop=(k == KH2 - 1))
                            for q, op in _COMBINE[p]:
                                if op is None:
                                    nc.scalar.activation(qt[q][:], mp[:], COPY)
                                else:
                                    nc.vector.tensor_tensor(qt[q][:], qt[q][:],
                                                            mp[:], op)
                        quads[gu] = qt
                    # h = silu(g) * u per quadrant -> bf16 into ht
                    for q in range(4):
                        row = i if q in (0, 2) else i + KIS2
                        col = slice(0, TH) if q in (0, 1) else slice(TH, S_TOK)
                        tmp = pB.tile([P, TH], F32, name=f"t{jtag}{i}q{q}",
                                      tag="silu", bufs=3)
                        nc.scalar.activation(tmp[:], quads["g"][q][:], SILU)
                        nc.vector.tensor_tensor(ht[:, row, col], tmp[:],
                                                quads["u"][q][:], MULT)

        with tc.tile_pool(name=f"shC{jtag}", bufs=1) as pC:
            D11 = ht[:, 0:KIS2, 0:TH]
            D12 = ht[:, KIS2:KIS, 0:TH]
            D21 = ht[:, 0:KIS2, TH:S_TOK]
            D22 = ht[:, KIS2:KIS, TH:S_TOK]
            dmv_specs = [(D11, D22, ADD), (D21, D22, ADD), None, None,
                         (D11, D12, ADD), (D21, D11, SUB), (D12, D22, SUB)]
            dmovs = []
            for p, spec in enumerate(dmv_specs):
                if spec is None:
                    dmovs.append(D11 if p == 2 else D22)
                    continue
                mt = pC.tile([P, KIS2, TH], BF16, name=f"dmv{jtag}{p}",
                             tag=f"dmv{p}")
                nc.vector.tensor_tensor(mt[:], spec[0], spec[1], spec[2])
                dmovs.append(mt)

            with tc.tile_pool(name=f"shCp{jtag}", bufs=1, space="PSUM") as pp:
                for m in range(KH2):
                    yq = [pC.tile([P, TH], F32, name=f"y{jtag}{m}q{q}",
                                  tag=f"yq{q}", bufs=2) for q in range(4)]
                    for p in range(7):
                        wt = pC.tile([P, KIS2, P], BF16,
                                     name=f"wd{jtag}{m}p{p}", tag=f"wd{p}",
                                     bufs=2)
                        nc.sync.dma_start(wt[:], wcd_ap[p, m])
                        mp = pp.tile([P, TH], F32, name=f"md{jtag}{m}p{p}",
                                     tag="md", bufs=6)
                        for k in range(KIS2):
                            nc.tensor.matmul(
                                mp[:], wt[:, k], dmovs[p][:, k, :],
                                start=(k == 0), stop=(k == KIS2 - 1))
                        for q, op in _COMBINE[p]:
                            if op is None:
                                nc.scalar.activation(yq[q][:], mp[:], COPY)
                            else:
                                nc.vector.tensor_tensor(yq[q][:], yq[q][:],
                                                        mp[:], op)
                    # C11=y[T1,H1] C12=y[T1,H2] C21=y[T2,H1] C22=y[T2,H2]
                    for q in range(4):
                        row = m if q in (0, 2) else m + KH2
                        c0 = 0 if q in (0, 1) else TH
                        nc.sync.dma_start(
                            out_ap[row * P:(row + 1) * P, c0:c0 + TH],
                            yq[q][:])


def _build(C, reps=1, loop=0):
    only = os.environ.get("MOE_ONLY", "")
    shared_mode = os.environ.get("MOE_SHARED", "strassen")
    routed_mode = os.environ.get("MOE_ROUTED", "bf16")
    nc = bacc.Bacc(trn_type="TRN2", target_bir_lowering=False, debug=False)
    xr = nc.dram_tensor("xr", [H, C], F32R, kind="ExternalInput")
    xrb = nc.dram_tensor("xrb", [P, KH, C], BF16, kind="ExternalInput")
    xs = nc.dram_tensor("xs", [P, KH, S_TOK], BF16, kind="ExternalInput")
    xsf = nc.dram_tensor("xsf", [H, S_TOK], F32R, kind="ExternalInput")
    w_in = {}
    for nm in ("rg", "ru", "ag", "au", "bg", "bu"):
        w_in[nm] = nc.dram_tensor(nm, [KI, P, KH, P], F32R,
                                  kind="ExternalInput")
    for nm in ("rd", "ad", "bd"):
        w_in[nm] = nc.dram_tensor(nm, [KH, P, KI, P], F32R,
                                  kind="ExternalInput")
    for nm in ("rgb", "rub"):
        w_in[nm] = nc.dram_tensor(nm, [KI, P, KH, P], BF16,
                                  kind="ExternalInput")
    w_in["rdb"] = nc.dram_tensor("rdb", [KH, P, KI, P], BF16,
                                 kind="ExternalInput")
    for nm in ("cg", "cu"):
        w_in[nm] = nc.dram_tensor(nm, [7, KIS // 2, P, KH // 2, P], BF16,
                                  kind="ExternalInput")
    w_in["cd"] = nc.dram_tensor("cd", [7, KH // 2, P, KIS // 2, P], BF16,
                                kind="ExternalInput")
    yr = nc.dram_tensor("yr", [H, C], F32, kind="ExternalOutput")
    ys = nc.dram_tensor("ys", [H, S_TOK], F32, kind="ExternalOutput")
    ysb = nc.dram_tensor("ysb", [H, S_TOK], F32, kind="ExternalOutput")

    with tile.TileContext(nc) as tc:
        def emit_routed(r):
            with (
                tc.tile_pool(name=f"sbufr{r}", bufs=2) as sbuf,
                tc.tile_pool(name=f"psumr{r}", bufs=2,
                             space="PSUM") as psum,
            ):
                if routed_mode == "bf16":
                    _emit_routed_bf16(nc, sbuf, psum, xrb.ap(),
                                      w_in["rgb"].ap(), w_in["rub"].ap(),
                                      w_in["rdb"].ap(), yr.ap(), C,
                                      f"r{r}_")
                else:
                    _emit_ffn_routed(nc, sbuf, psum, xr.ap(),
                                     w_in["rg"].ap(), w_in["ru"].ap(),
                                     w_in["rd"].ap(), yr.ap(), C,
                                     f"r{r}_")

        def emit_shared(r):
            if shared_mode == "strassen":
                _emit_shared_strassen(
                    nc, tc, xs.ap(), w_in["cg"].ap(), w_in["cu"].ap(),
                    w_in["cd"].ap(), ys.ap(), f"s{r}_")
            else:
                with (
                    tc.tile_pool(name=f"sbufs{r}", bufs=2) as sbuf,
                    tc.tile_pool(name=f"psums{r}", bufs=2,
                                 space="PSUM") as psum,
                ):
                    _emit_ffn_routed(
                        nc, sbuf, psum, xsf.ap(), w_in["ag"].ap(),
                        w_in["au"].ap(), w_in["ad"].ap(), ys.ap(),
                        S_TOK, f"a{r}_")
                    _emit_ffn_routed(
                        nc, sbuf, psum, xsf.ap(), w_in["bg"].ap(),
                        w_in["bu"].ap(), w_in["bd"].ap(), ysb.ap(),
                        S_TOK, f"b{r}_")

        def body(r):
            jobs = []
            if only != "shared":
                jobs.append(emit_routed)
            if only != "routed":
                jobs.append(emit_shared)
            if os.environ.get("MOE_ORDER", "") == "shared_first":
                jobs.reverse()
            for j in jobs:
                j(r)

        if loop:
            with tc.For_i(0, loop, 1):
                for r in range(reps):
                    body(r)
        else:
            for r in range(reps):
                body(r)
    nc.compile()
    return nc


def _get_nc(C, reps=1, loop=0):
    key = (C, reps, loop, os.environ.get("MOE_ONLY", ""),
           os.environ.get("MOE_SHARED", "strassen"),
           os.environ.get("MOE_ROUTED", "bf16"),
           os.environ.get("MOE_ORDER", ""))
    if key not in _BUILD_CACHE:
        _BUILD_CACHE[key] = _build(C, reps, loop)
    return _BUILD_CACHE[key]


def _gate_host(x, gate_w):
    """Softmax + top-2 + renormalize, mirroring the jax reference on CPU."""
    try:
        import jax
        import jax.numpy as jnp
        cpu = jax.devices("cpu")[0]
        with jax.default_device(cpu):
            logits = jnp.asarray(x) @ jnp.asarray(gate_w).T
            scores = jax.nn.softmax(logits, axis=-1)
            topk_w, topk_idx = jax.lax.top_k(scores, TOP_K)
            topk_w = topk_w / (jnp.sum(topk_w, axis=-1, keepdims=True) + 1e-20)
        return np.asarray(topk_w), np.asarray(topk_idx)
    except Exception:
        logits = x @ gate_w.T
        m = logits.max(axis=-1, keepdims=True)
        ex = np.exp(logits - m)
        scores = ex / ex.sum(axis=-1, keepdims=True)
        order = np.argsort(-scores, axis=-1, kind="stable")
        topk_idx = order[:, :TOP_K]
        topk_w = np.take_along_axis(scores, topk_idx, axis=-1)
        topk_w = topk_w / (topk_w.sum(axis=-1, keepdims=True) + 1e-20)
        return topk_w.astype(np.float32), topk_idx.astype(np.int32)


def _wlayout_ud(w):
    # [H, I_like] -> [KI', P(H), KH, P(I)]  (stationary tiles for up/gate)
    ki = w.shape[1] // P
    return np.ascontiguousarray(w.reshape(KH, P, ki, P).transpose(2, 1, 0, 3))


def _wlayout_down(w):
    # [I_like, H] -> [KH, P(I), KI', P(H)]
    ki = w.shape[0] // P
    return np.ascontiguousarray(w.reshape(ki, P, KH, P).transpose(2, 1, 0, 3))


def _strassen_combos(B11, B12, B21, B22):
    return [B11 + B22, B11, B12 - B22, B21 - B11, B22, B11 + B12, B21 + B22]


def _wlayout_strassen_ud(w):
    """[H, IS] -> [7, KIS/2, P, KH/2, P] bf16 weight-combo tensor."""
    h2, n2 = H // 2, IS // 2
    combos = _strassen_combos(w[:h2, :n2], w[:h2, n2:], w[h2:, :n2],
                              w[h2:, n2:])
    out = np.empty((7, n2 // P, P, h2 // P, P), dtype=BF)
    for p, cb in enumerate(combos):
        out[p] = cb.reshape(h2 // P, P, n2 // P, P).transpose(
            2, 1, 0, 3).astype(BF)
    return np.ascontiguousarray(out)


def _wlayout_strassen_down(w):
    """[IS, H] -> [7, KH/2, P, KIS/2, P] bf16 weight-combo tensor."""
    k2, m2 = IS // 2, H // 2
    combos = _strassen_combos(w[:k2, :m2], w[:k2, m2:], w[k2:, :m2],
                              w[k2:, m2:])
    out = np.empty((7, m2 // P, P, k2 // P, P), dtype=BF)
    for p, cb in enumerate(combos):
        out[p] = cb.reshape(k2 // P, P, m2 // P, P).transpose(
            2, 1, 0, 3).astype(BF)
    return np.ascontiguousarray(out)


def _prepare(hidden_states, gate_w, we_gate, we_up, we_down,
             ws_gate, ws_up, ws_down):
    B, S, h = hidden_states.shape
    x = np.ascontiguousarray(hidden_states.reshape(-1, h))  # [T, H]

    topk_w, topk_idx = _gate_host(x, gate_w)

    idx_lists, w_lists = [], []
    for e in range(E):
        mask = (topk_idx == e)
        idx = np.nonzero(mask.any(axis=1))[0]
        we = np.where(mask, topk_w, 0.0).sum(axis=1)[idx].astype(np.float32)
        idx_lists.append(idx)
        w_lists.append(we)
    C = max(512, (max(len(ix) for ix in idx_lists) + 3) & ~3)

    cg = _wlayout_strassen_ud(ws_gate)
    cu = _wlayout_strassen_ud(ws_up)
    cd = _wlayout_strassen_down(ws_down)
    ag = _wlayout_ud(ws_gate[:, :I])
    bg = _wlayout_ud(ws_gate[:, I:])
    au = _wlayout_ud(ws_up[:, :I])
    bu = _wlayout_ud(ws_up[:, I:])
    ad = _wlayout_down(ws_down[:I])
    bd = _wlayout_down(ws_down[I:])

    in_maps = []
    for c in range(E):
        idx = idx_lists[c]
        xr = np.zeros((H, C), dtype=np.float32)
        xr[:, :len(idx)] = x[idx].T
        xrb = np.ascontiguousarray(
            xr.reshape(KH, P, C).transpose(1, 0, 2).astype(BF))
        xsf = np.ascontiguousarray(x[c * S_TOK:(c + 1) * S_TOK].T)
        xs = np.ascontiguousarray(
            xsf.reshape(KH, P, S_TOK).transpose(1, 0, 2).astype(BF))
        rg = _wlayout_ud(we_gate[c])
        ru = _wlayout_ud(we_up[c])
        rd = _wlayout_down(we_down[c])
        in_maps.append({
            "xr": xr, "xrb": xrb, "xs": xs, "xsf": xsf,
            "rg": rg, "ru": ru, "rd": rd,
            "rgb": rg.astype(BF), "rub": ru.astype(BF),
            "rdb": rd.astype(BF),
            "cg": cg, "cu": cu, "cd": cd,
            "ag": ag, "au": au, "ad": ad,
            "bg": bg, "bu": bu, "bd": bd,
        })
    return in_maps, idx_lists, w_lists, C


def _combine(results, idx_lists, w_lists, T):
    direct = os.environ.get("MOE_SHARED", "strassen") == "direct"
    y = np.empty((T, H), dtype=np.float32)
    for c in range(E):
        sh = results[c]["ys"]
        if direct:
            sh = sh + results[c]["ysb"]
        y[c * S_TOK:(c + 1) * S_TOK] = sh.T
    for c in range(E):
        idx = idx_lists[c]
        y[idx] += w_lists[c][:, None] * results[c]["yr"][:, :len(idx)].T
    return y


def kernel(hidden_states, gate_w, we_gate, we_up, we_down,
           ws_gate, ws_up, ws_down):
    global LAST_RESULTS
    B, S, h = hidden_states.shape
    in_maps, idx_lists, w_lists, C = _prepare(
        hidden_states, gate_w, we_gate, we_up, we_down,
        ws_gate, ws_up, ws_down)

    nc = _get_nc(C)

    trace_env = os.environ.get("MOE_TRACE", "")
    kwargs = {}
    if trace_env:
        kwargs["trace"] = True
        kwargs["trace_cores"] = [int(t) for t in trace_env.split(",")] \
            if trace_env != "1" else [0]
    res = run_bass_kernel_spmd(nc, in_maps, core_ids=list(range(E)), **kwargs)
    LAST_RESULTS = res

    y = _combine(res.results, idx_lists, w_lists, B * S)
    return y.reshape(B, S, h)
